# revision 7
# baseline (speedup 1.0000x reference)
"""Trainium2 Bass kernel for AssociativeScanGLRU.

Reference computation (per batch b, channel c, time t):
    inp  = tanh(x[:, :, :1024])
    i    = sigmoid(x[:, :, 1024:2048])      # input gate
    og   = sigmoid(x[:, :, 2048:3072])      # output gate
    f    = 1 - i                            # forget gate (tied)
    s    = inp * i;  h_{-1} = carry
    h_t  = f_t * h_{t-1} + s_t              # linear scan over t
    y    = tanh(h) * og;  h_last = h[:, -1, :]

Sharding: 8 cores = batch(4) x d_h-halves(2); no cross-core communication.
Each core handles 512 channels x 4096 timesteps.

Per-core dataflow (time chunks of TC=512):
  DMA x_in/x_ig with f32->bf16 cast (SWDGE) in T-layout (time on
  partitions); x_og f32 via HWDGE
  -> sigmoid(og) in T-layout on ScalarE
  -> TensorE transposes bf16 x_in / x_ig into C-layout (channels on
     partitions) PSUM tiles (bf16: FWL weight loads, 2-bank tiles)
  -> tanh / sigmoid read PSUM -> f32 SBUF on ScalarE
  -> VectorE: f = 1-i (tensor_scalar), s = tanh_in*i (tensor_tensor),
     h = tensor_tensor_scan(f, s) -- the native HW linear scan (fp32
     internal state, bf16 output)
  -> TensorE transposes h back to T-layout PSUM
  -> tanh(h) on ScalarE, y = th*og on VectorE, DMA out f32.
"""

import numpy as np

import concourse.bass as bass
import concourse.mybir as mybir
import concourse.tile as tile
from concourse import bacc, masks
from concourse.bass_utils import run_bass_kernel_spmd

F32 = mybir.dt.float32
BF16 = mybir.dt.bfloat16
AF = mybir.ActivationFunctionType
OP = mybir.AluOpType

B = 4
S = 4096
DH = 1024
DHC = 512          # channels per core (d_h half)
TC = 512           # time chunk
NCH = S // TC      # 8 chunks
P = 128
TT = TC // P       # 4 time subtiles per chunk
CB = DHC // P      # 4 channel blocks per core
FREE = TT * DHC    # 2048 free elements per big tile


def _build():
    nc = bacc.Bacc("TRN2", target_bir_lowering=False, debug=False)
    xin = nc.declare_dram_parameter("xin", [S, DHC], F32, isOutput=False)
    xig = nc.declare_dram_parameter("xig", [S, DHC], F32, isOutput=False)
    xog = nc.declare_dram_parameter("xog", [S, DHC], F32, isOutput=False)
    carry = nc.declare_dram_parameter("carry", [CB, P], F32, isOutput=False)
    y = nc.declare_dram_parameter("y", [S, DHC], F32, isOutput=True)
    h_last = nc.declare_dram_parameter("h_last", [CB, P], F32, isOutput=True)

    with tile.TileContext(nc) as tc:
        with (
            tc.tile_pool(name="const", bufs=1) as const_pool,
            tc.tile_pool(name="tin", bufs=3) as tin_pool,
            tc.tile_pool(name="cbuf", bufs=3) as c_pool,
            tc.tile_pool(name="ps", bufs=4, space="PSUM") as ps_pool,
        ):
            ident = const_pool.tile([P, P], BF16)
            masks.make_identity(nc, ident[:])

            # carry (CB, P) f32 DRAM -> bf16 SBUF (cast DMA) -> transpose
            carry_in = const_pool.tile([CB, P], BF16)
            nc.gpsimd.dma_start(carry_in[:], carry[:, :])
            carry_ps = ps_pool.tile([P, CB], BF16, tag="ps")
            nc.tensor.transpose(carry_ps[:], carry_in[:], ident[0:CB, 0:CB])
            carry_sb = const_pool.tile([P, CB], F32)
            nc.vector.tensor_copy(carry_sb[:], carry_ps[:])

            h_prev = None
            for k in range(NCH):
                trows = slice(k * TC, (k + 1) * TC)
                # ---- DMA in, T-layout: partition p = t_lo, free = (tt, c)
                # x_in / x_ig: f32 -> bf16 cast during DMA (SWDGE)
                xin_t = tin_pool.tile([P, FREE], BF16, tag="xin")
                nc.gpsimd.dma_start(
                    xin_t[:].rearrange("p (tt c) -> p tt c", tt=TT),
                    xin[trows, :].rearrange("(tt p) c -> p tt c", p=P),
                )
                xig_t = tin_pool.tile([P, FREE], BF16, tag="xig")
                nc.gpsimd.dma_start(
                    xig_t[:].rearrange("p (tt c) -> p tt c", tt=TT),
                    xig[trows, :].rearrange("(tt p) c -> p tt c", p=P),
                )
                xog_t = tin_pool.tile([P, FREE], F32, tag="xog")
                nc.sync.dma_start(
                    xog_t[:].rearrange("p (tt c) -> p tt c", tt=TT),
                    xog[trows, :].rearrange("(tt p) c -> p tt c", p=P),
                )

                # og = sigmoid(xog) -> bf16, T-layout
                og_t = tin_pool.tile([P, FREE], BF16, tag="og")
                nc.scalar.activation(og_t[:], xog_t[:], AF.Sigmoid)

                # ---- PE transposes of bf16 xin / xig into C-layout PSUM
                # C-layout free index = cb*TC + t_in_chunk (t = tt*128 + p_t)
                in_ps = ps_pool.tile([P, FREE], BF16, tag="ps")
                ig_ps = ps_pool.tile([P, FREE], BF16, tag="ps")
                for cb in range(CB):
                    for tt in range(TT):
                        c0 = tt * DHC + cb * P
                        dst = slice(cb * TC + tt * P, cb * TC + (tt + 1) * P)
                        for ps, xt in ((in_ps, xin_t), (ig_ps, xig_t)):
                            for j in range(4):
                                nc.tensor.transpose(
                                    ps[32 * j : 32 * (j + 1), dst],
                                    xt[:, c0 + 32 * j : c0 + 32 * (j + 1)],
                                    ident[:],
                                    tile_position=(0, 32 * j),
                                )

                # ---- activations PSUM -> SBUF (C-layout, f32)
                tanh_in = c_pool.tile([P, FREE], BF16, tag="tanh_in")
                nc.scalar.activation(tanh_in[:], in_ps[:], AF.Tanh)
                i_c = c_pool.tile([P, FREE], BF16, tag="i_c")
                nc.scalar.activation(i_c[:], ig_ps[:], AF.Sigmoid)

                # ---- vector: f = 1 - i ; s = tanh_in * i (in place over i_c)
                f_c = c_pool.tile([P, FREE], BF16, tag="f_c")
                nc.vector.tensor_scalar(f_c[:], i_c[:], -1.0, 1.0, OP.mult, OP.add)
                nc.vector.tensor_tensor(i_c[:], tanh_in[:], i_c[:], OP.mult)

                # ---- the scan: h = f * h_prev + s, per channel block
                # fp32 internal state, bf16 output (for cheap transposes)
                h_c = c_pool.tile([P, FREE], BF16, tag="h_c")
                for cb in range(CB):
                    cs = slice(cb * TC, (cb + 1) * TC)
                    if k == 0:
                        init = carry_sb[:, cb : cb + 1]
                    else:
                        init = h_prev[:, cb * TC + TC - 1 : cb * TC + TC]
                    nc.vector.tensor_tensor_scan(
                        h_c[:, cs], f_c[:, cs], i_c[:, cs], init, OP.mult, OP.add
                    )

                # ---- transpose h back to T-layout PSUM (bf16)
                h_ps = ps_pool.tile([P, FREE], BF16, tag="ps")
                for cb in range(CB):
                    for tt in range(TT):
                        t0 = cb * TC + tt * P
                        dst = slice(tt * DHC + cb * P, tt * DHC + (cb + 1) * P)
                        for j in range(4):
                            nc.tensor.transpose(
                                h_ps[32 * j : 32 * (j + 1), dst],
                                h_c[:, t0 + 32 * j : t0 + 32 * (j + 1)],
                                ident[:],
                                tile_position=(0, 32 * j),
                            )

                # ---- y = tanh(h) * og, T-layout bf16; cast-DMA out f32
                th_t = c_pool.tile([P, FREE], BF16, tag="th_t")
                nc.scalar.activation(th_t[:], h_ps[:], AF.Tanh)
                nc.vector.tensor_tensor(th_t[:], th_t[:], og_t[:], OP.mult)
                nc.gpsimd.dma_start(
                    y[trows, :].rearrange("(tt p) c -> p tt c", p=P),
                    th_t[:].rearrange("p (tt c) -> p tt c", tt=TT),
                )

                if k == NCH - 1:
                    # h_last = h[:, -1] per channel block: (P, CB) strided AP
                    hl_ps = ps_pool.tile([CB, P], BF16, tag="ps")
                    nc.tensor.transpose(hl_ps[:], h_c[:, TC - 1 :: TC], ident[:])
                    hl_sb = const_pool.tile([CB, P], F32)
                    nc.scalar.copy(hl_sb[:], hl_ps[:])
                    nc.sync.dma_start(h_last[:, :], hl_sb[:])

                h_prev = h_c

    nc.compile()
    return nc


def _in_maps(x, carry):
    x = np.asarray(x, dtype=np.float32)
    carry = np.asarray(carry, dtype=np.float32)
    maps = []
    for core in range(8):
        b, half = core // 2, core % 2
        c0 = half * DHC
        maps.append(
            {
                "xin": np.ascontiguousarray(x[b, :, c0 : c0 + DHC]),
                "xig": np.ascontiguousarray(x[b, :, DH + c0 : DH + c0 + DHC]),
                "xog": np.ascontiguousarray(
                    x[b, :, 2 * DH + c0 : 2 * DH + c0 + DHC]
                ),
                "carry": np.ascontiguousarray(
                    carry[b, c0 : c0 + DHC].reshape(CB, P)
                ),
            }
        )
    return maps


def run(x, carry, trace=False, **kwargs):
    nc = _build()
    res = run_bass_kernel_spmd(
        nc, _in_maps(x, carry), core_ids=list(range(8)), trace=trace, **kwargs
    )
    y = np.zeros((B, S, DH), np.float32)
    hl = np.zeros((B, DH), np.float32)
    for core in range(8):
        b, half = core // 2, core % 2
        c0 = half * DHC
        y[b, :, c0 : c0 + DHC] = res.results[core]["y"]
        hl[b, c0 : c0 + DHC] = res.results[core]["h_last"].reshape(DHC)
    return (hl, y), res


def kernel(x, carry):
    out, _ = run(x, carry)
    return out


# revision 8
# speedup vs baseline: 1.7437x; 1.7437x over previous
"""Trainium2 Bass kernel for AssociativeScanGLRU.

Reference computation (per batch b, channel c, time t):
    inp  = tanh(x[:, :, :1024])
    i    = sigmoid(x[:, :, 1024:2048])      # input gate
    og   = sigmoid(x[:, :, 2048:3072])      # output gate
    f    = 1 - i                            # forget gate (tied)
    s    = inp * i;  h_{-1} = carry
    h_t  = f_t * h_{t-1} + s_t              # linear scan over t
    y    = tanh(h) * og;  h_last = h[:, -1, :]

Sharding: 8 cores = batch(4) x d_h-halves(2); no cross-core communication.
Each core handles 512 channels x 4096 timesteps.

Per-core dataflow (time chunks of TC=512):
  DMA x_in/x_ig with f32->bf16 cast (SWDGE) in T-layout (time on
  partitions); x_og f32 via HWDGE
  -> sigmoid(og) in T-layout on ScalarE
  -> TensorE transposes bf16 x_in / x_ig into C-layout (channels on
     partitions) PSUM tiles (bf16: FWL weight loads, 2-bank tiles)
  -> tanh / sigmoid read PSUM -> f32 SBUF on ScalarE
  -> VectorE: f = 1-i (tensor_scalar), s = tanh_in*i (tensor_tensor),
     h = tensor_tensor_scan(f, s) -- the native HW linear scan (fp32
     internal state, bf16 output)
  -> TensorE transposes h back to T-layout PSUM
  -> tanh(h) on ScalarE, y = th*og on VectorE, DMA out f32.
"""

import numpy as np

import concourse.bass as bass
import concourse.mybir as mybir
import concourse.tile as tile
from concourse import bacc, masks
from concourse.bass_utils import run_bass_kernel_spmd

F32 = mybir.dt.float32
BF16 = mybir.dt.bfloat16
AF = mybir.ActivationFunctionType
OP = mybir.AluOpType

B = 4
S = 4096
DH = 1024
DHC = 512          # channels per core (d_h half)
TC = 512           # time chunk
NCH = S // TC      # 8 chunks
P = 128
TT = TC // P       # 4 time subtiles per chunk
CB = DHC // P      # 4 channel blocks per core
FREE = TT * DHC    # 2048 free elements per big tile


def _build():
    nc = bacc.Bacc("TRN2", target_bir_lowering=False, debug=False)
    xin = nc.declare_dram_parameter("xin", [S, DHC], F32, isOutput=False)
    xig = nc.declare_dram_parameter("xig", [S, DHC], F32, isOutput=False)
    xog = nc.declare_dram_parameter("xog", [S, DHC], F32, isOutput=False)
    carry = nc.declare_dram_parameter("carry", [CB, P], F32, isOutput=False)
    y = nc.declare_dram_parameter("y", [S, DHC], F32, isOutput=True)
    h_last = nc.declare_dram_parameter("h_last", [CB, P], F32, isOutput=True)

    with tile.TileContext(nc) as tc:
        with (
            tc.tile_pool(name="const", bufs=1) as const_pool,
            tc.tile_pool(name="tin", bufs=4) as tin_pool,
            tc.tile_pool(name="cbuf", bufs=3) as c_pool,
            tc.tile_pool(name="ps", bufs=4, space="PSUM") as ps_pool,
        ):
            ident = const_pool.tile([P, P], BF16)
            masks.make_identity(nc, ident[:])

            # carry (CB, P) f32 DRAM -> bf16 SBUF (cast DMA) -> transpose
            carry_in = const_pool.tile([CB, P], BF16)
            nc.gpsimd.dma_start(carry_in[:], carry[:, :])
            carry_ps = ps_pool.tile([P, CB], BF16, tag="ps")
            nc.tensor.transpose(carry_ps[:], carry_in[:], ident[0:CB, 0:CB])
            carry_sb = const_pool.tile([P, CB], F32)
            nc.vector.tensor_copy(carry_sb[:], carry_ps[:])

            h_prev = None
            for k in range(NCH):
                trows = slice(k * TC, (k + 1) * TC)
                # ---- DMA in, T-layout: partition p = t_lo, free = (tt, c)
                # x_in / x_ig: f32 -> bf16 cast during DMA (SWDGE)
                xin_t = tin_pool.tile([P, FREE], BF16, tag="xin")
                nc.gpsimd.dma_start(
                    xin_t[:].rearrange("p (tt c) -> p tt c", tt=TT),
                    xin[trows, :].rearrange("(tt p) c -> p tt c", p=P),
                )
                xig_t = tin_pool.tile([P, FREE], BF16, tag="xig")
                nc.gpsimd.dma_start(
                    xig_t[:].rearrange("p (tt c) -> p tt c", tt=TT),
                    xig[trows, :].rearrange("(tt p) c -> p tt c", p=P),
                )
                xog_t = tin_pool.tile([P, FREE], F32, tag="xog")
                nc.sync.dma_start(
                    xog_t[:].rearrange("p (tt c) -> p tt c", tt=TT),
                    xog[trows, :].rearrange("(tt p) c -> p tt c", p=P),
                )

                # og = sigmoid(xog) -> bf16, T-layout
                og_t = tin_pool.tile([P, FREE], BF16, tag="og")
                nc.scalar.activation(og_t[:], xog_t[:], AF.Sigmoid)

                # ---- PE transposes of bf16 xin / xig into C-layout PSUM
                # C-layout free index = cb*TC + t_in_chunk (t = tt*128 + p_t)
                in_ps = ps_pool.tile([P, FREE], BF16, tag="ps")
                ig_ps = ps_pool.tile([P, FREE], BF16, tag="ps")
                for cb in range(CB):
                    for tt in range(TT):
                        src = slice(tt * DHC + cb * P, tt * DHC + (cb + 1) * P)
                        dst = slice(cb * TC + tt * P, cb * TC + (tt + 1) * P)
                        nc.tensor.transpose(in_ps[:, dst], xin_t[:, src], ident[:])
                        nc.tensor.transpose(ig_ps[:, dst], xig_t[:, src], ident[:])

                # ---- activations PSUM -> SBUF (C-layout, f32)
                tanh_in = c_pool.tile([P, FREE], BF16, tag="tanh_in")
                nc.scalar.activation(tanh_in[:], in_ps[:], AF.Tanh)
                i_c = c_pool.tile([P, FREE], BF16, tag="i_c")
                nc.scalar.activation(i_c[:], ig_ps[:], AF.Sigmoid)

                # ---- vector: f = 1 - i ; s = tanh_in * i (in place over i_c)
                f_c = c_pool.tile([P, FREE], BF16, tag="f_c")
                nc.vector.tensor_scalar(f_c[:], i_c[:], -1.0, 1.0, OP.mult, OP.add)
                nc.vector.tensor_tensor(i_c[:], tanh_in[:], i_c[:], OP.mult)

                # ---- the scan: h = f * h_prev + s, per channel block
                # fp32 internal state, bf16 output (for cheap transposes)
                h_c = c_pool.tile([P, FREE], BF16, tag="h_c")
                for cb in range(CB):
                    cs = slice(cb * TC, (cb + 1) * TC)
                    if k == 0:
                        init = carry_sb[:, cb : cb + 1]
                    else:
                        init = h_prev[:, cb * TC + TC - 1 : cb * TC + TC]
                    nc.vector.tensor_tensor_scan(
                        h_c[:, cs], f_c[:, cs], i_c[:, cs], init, OP.mult, OP.add
                    )

                # ---- transpose h back to T-layout PSUM (bf16)
                h_ps = ps_pool.tile([P, FREE], BF16, tag="ps")
                for cb in range(CB):
                    for tt in range(TT):
                        src = slice(cb * TC + tt * P, cb * TC + (tt + 1) * P)
                        dst = slice(tt * DHC + cb * P, tt * DHC + (cb + 1) * P)
                        nc.tensor.transpose(h_ps[:, dst], h_c[:, src], ident[:])

                # ---- y = tanh(h) * og, T-layout bf16; cast-DMA out f32
                th_t = c_pool.tile([P, FREE], BF16, tag="th_t")
                nc.scalar.activation(th_t[:], h_ps[:], AF.Tanh)
                nc.vector.tensor_tensor(th_t[:], th_t[:], og_t[:], OP.mult)
                nc.gpsimd.dma_start(
                    y[trows, :].rearrange("(tt p) c -> p tt c", p=P),
                    th_t[:].rearrange("p (tt c) -> p tt c", tt=TT),
                )

                if k == NCH - 1:
                    # h_last = h[:, -1] per channel block: (P, CB) strided AP
                    hl_ps = ps_pool.tile([CB, P], BF16, tag="ps")
                    nc.tensor.transpose(hl_ps[:], h_c[:, TC - 1 :: TC], ident[:])
                    hl_sb = const_pool.tile([CB, P], F32)
                    nc.scalar.copy(hl_sb[:], hl_ps[:])
                    nc.sync.dma_start(h_last[:, :], hl_sb[:])

                h_prev = h_c

    nc.compile()
    return nc


def _in_maps(x, carry):
    x = np.asarray(x, dtype=np.float32)
    carry = np.asarray(carry, dtype=np.float32)
    maps = []
    for core in range(8):
        b, half = core // 2, core % 2
        c0 = half * DHC
        maps.append(
            {
                "xin": np.ascontiguousarray(x[b, :, c0 : c0 + DHC]),
                "xig": np.ascontiguousarray(x[b, :, DH + c0 : DH + c0 + DHC]),
                "xog": np.ascontiguousarray(
                    x[b, :, 2 * DH + c0 : 2 * DH + c0 + DHC]
                ),
                "carry": np.ascontiguousarray(
                    carry[b, c0 : c0 + DHC].reshape(CB, P)
                ),
            }
        )
    return maps


def run(x, carry, trace=False, **kwargs):
    nc = _build()
    res = run_bass_kernel_spmd(
        nc, _in_maps(x, carry), core_ids=list(range(8)), trace=trace, **kwargs
    )
    y = np.zeros((B, S, DH), np.float32)
    hl = np.zeros((B, DH), np.float32)
    for core in range(8):
        b, half = core // 2, core % 2
        c0 = half * DHC
        y[b, :, c0 : c0 + DHC] = res.results[core]["y"]
        hl[b, c0 : c0 + DHC] = res.results[core]["h_last"].reshape(DHC)
    return (hl, y), res


def kernel(x, carry):
    out, _ = run(x, carry)
    return out


# revision 9
# speedup vs baseline: 2.0260x; 1.1619x over previous
"""Trainium2 Bass kernel for AssociativeScanGLRU.

Reference computation (per batch b, channel c, time t):
    inp  = tanh(x[:, :, :1024])
    i    = sigmoid(x[:, :, 1024:2048])      # input gate
    og   = sigmoid(x[:, :, 2048:3072])      # output gate
    f    = 1 - i                            # forget gate (tied)
    s    = inp * i;  h_{-1} = carry
    h_t  = f_t * h_{t-1} + s_t              # linear scan over t
    y    = tanh(h) * og;  h_last = h[:, -1, :]

Sharding: 8 cores = batch(4) x d_h-halves(2); no cross-core communication.
Each core handles 512 channels x 4096 timesteps.

Per-core dataflow (time chunks of TC=512):
  DMA x_in/x_ig with f32->bf16 cast (SWDGE) in T-layout (time on
  partitions); x_og f32 via HWDGE
  -> sigmoid(og) in T-layout on ScalarE
  -> TensorE transposes bf16 x_in / x_ig into C-layout (channels on
     partitions) PSUM tiles (bf16: FWL weight loads, 2-bank tiles)
  -> tanh / sigmoid read PSUM -> f32 SBUF on ScalarE
  -> VectorE: f = 1-i (tensor_scalar), s = tanh_in*i (tensor_tensor),
     h = tensor_tensor_scan(f, s) -- the native HW linear scan (fp32
     internal state, bf16 output)
  -> TensorE transposes h back to T-layout PSUM
  -> tanh(h) on ScalarE, y = th*og on VectorE, DMA out f32.
"""

import numpy as np

import concourse.bass as bass
import concourse.mybir as mybir
import concourse.tile as tile
from concourse import bacc, masks
from concourse.bass_utils import run_bass_kernel_spmd

F32 = mybir.dt.float32
BF16 = mybir.dt.bfloat16
AF = mybir.ActivationFunctionType
OP = mybir.AluOpType

B = 4
S = 4096
DH = 1024
DHC = 512          # channels per core (d_h half)
TC = 512           # time chunk
NCH = S // TC      # 8 chunks
P = 128
TT = TC // P       # 4 time subtiles per chunk
CB = DHC // P      # 4 channel blocks per core
FREE = TT * DHC    # 2048 free elements per big tile


def _build():
    nc = bacc.Bacc("TRN2", target_bir_lowering=False, debug=False)
    xin = nc.declare_dram_parameter("xin", [S, DHC], F32, isOutput=False)
    xig = nc.declare_dram_parameter("xig", [S, DHC], F32, isOutput=False)
    xog = nc.declare_dram_parameter("xog", [S, DHC], F32, isOutput=False)
    carry = nc.declare_dram_parameter("carry", [CB, P], F32, isOutput=False)
    y = nc.declare_dram_parameter("y", [S, DHC], F32, isOutput=True)
    h_last = nc.declare_dram_parameter("h_last", [CB, P], F32, isOutput=True)

    with tile.TileContext(nc) as tc:
        with (
            tc.tile_pool(name="const", bufs=1) as const_pool,
            tc.tile_pool(name="tin", bufs=4) as tin_pool,
            tc.tile_pool(name="cbuf", bufs=3) as c_pool,
            tc.tile_pool(name="ps", bufs=4, space="PSUM") as ps_pool,
        ):
            ident = const_pool.tile([P, P], BF16)
            masks.make_identity(nc, ident[:])

            # carry (CB, P) f32 DRAM -> bf16 SBUF (cast DMA) -> transpose
            carry_in = const_pool.tile([CB, P], BF16)
            nc.gpsimd.dma_start(carry_in[:], carry[:, :])
            carry_ps = ps_pool.tile([P, CB], BF16, tag="ps")
            nc.tensor.transpose(carry_ps[:], carry_in[:], ident[0:CB, 0:CB])
            carry_sb = const_pool.tile([P, CB], F32)
            nc.vector.tensor_copy(carry_sb[:], carry_ps[:])

            def emit_tail(k, h_c, og_t):
                """h-back transpose + tanh + y-mul + DMA-out for chunk k.

                Emitted one iteration late (software pipelining): the PE
                executes in program order, so placing chunk k's h-back
                transposes AFTER chunk k+1's forward transposes lets the
                PE work on k+1's transposes while chunk k's scans finish
                (instead of idling ~7us every chunk, which also
                re-throttles the HAM clock gate)."""
                trows = slice(k * TC, (k + 1) * TC)
                h_ps = ps_pool.tile([P, FREE], BF16, tag="ps")
                for cb in range(CB):
                    for tt in range(TT):
                        src = slice(cb * TC + tt * P, cb * TC + (tt + 1) * P)
                        dst = slice(tt * DHC + cb * P, tt * DHC + (cb + 1) * P)
                        nc.tensor.transpose(h_ps[:, dst], h_c[:, src], ident[:])

                th_t = c_pool.tile([P, FREE], BF16, tag="th_t")
                nc.scalar.activation(th_t[:], h_ps[:], AF.Tanh)
                nc.vector.tensor_tensor(th_t[:], th_t[:], og_t[:], OP.mult)
                nc.gpsimd.dma_start(
                    y[trows, :].rearrange("(tt p) c -> p tt c", p=P),
                    th_t[:].rearrange("p (tt c) -> p tt c", tt=TT),
                )

            h_prev = None
            prev_tail = None  # (k, h_c, og_t) pending emit_tail
            for k in range(NCH):
                trows = slice(k * TC, (k + 1) * TC)
                # ---- DMA in, T-layout: partition p = t_lo, free = (tt, c)
                # x_in / x_ig: f32 -> bf16 cast during DMA (SWDGE)
                xin_t = tin_pool.tile([P, FREE], BF16, tag="xin")
                nc.gpsimd.dma_start(
                    xin_t[:].rearrange("p (tt c) -> p tt c", tt=TT),
                    xin[trows, :].rearrange("(tt p) c -> p tt c", p=P),
                )
                xig_t = tin_pool.tile([P, FREE], BF16, tag="xig")
                nc.gpsimd.dma_start(
                    xig_t[:].rearrange("p (tt c) -> p tt c", tt=TT),
                    xig[trows, :].rearrange("(tt p) c -> p tt c", p=P),
                )
                xog_t = tin_pool.tile([P, FREE], F32, tag="xog")
                nc.sync.dma_start(
                    xog_t[:].rearrange("p (tt c) -> p tt c", tt=TT),
                    xog[trows, :].rearrange("(tt p) c -> p tt c", p=P),
                )

                # og = sigmoid(xog) -> bf16, T-layout
                og_t = tin_pool.tile([P, FREE], BF16, tag="og")
                nc.scalar.activation(og_t[:], xog_t[:], AF.Sigmoid)

                # ---- PE transposes of bf16 xin / xig into C-layout PSUM
                # C-layout free index = cb*TC + t_in_chunk (t = tt*128 + p_t)
                in_ps = ps_pool.tile([P, FREE], BF16, tag="ps")
                ig_ps = ps_pool.tile([P, FREE], BF16, tag="ps")
                for cb in range(CB):
                    for tt in range(TT):
                        src = slice(tt * DHC + cb * P, tt * DHC + (cb + 1) * P)
                        dst = slice(cb * TC + tt * P, cb * TC + (tt + 1) * P)
                        nc.tensor.transpose(in_ps[:, dst], xin_t[:, src], ident[:])
                        nc.tensor.transpose(ig_ps[:, dst], xig_t[:, src], ident[:])

                # ---- previous chunk's h-back transpose + output path, now
                # that this chunk's forward transposes are queued on the PE
                if prev_tail is not None:
                    emit_tail(*prev_tail)

                # ---- activations PSUM -> SBUF (C-layout)
                tanh_in = c_pool.tile([P, FREE], BF16, tag="tanh_in")
                nc.scalar.activation(tanh_in[:], in_ps[:], AF.Tanh)
                i_c = c_pool.tile([P, FREE], BF16, tag="i_c")
                nc.scalar.activation(i_c[:], ig_ps[:], AF.Sigmoid)

                # ---- vector: f = 1 - i ; s = tanh_in * i (in place over i_c)
                f_c = c_pool.tile([P, FREE], BF16, tag="f_c")
                nc.vector.tensor_scalar(f_c[:], i_c[:], -1.0, 1.0, OP.mult, OP.add)
                nc.vector.tensor_tensor(i_c[:], tanh_in[:], i_c[:], OP.mult)

                # ---- the scan: h = f * h_prev + s, per channel block
                # fp32 internal state, bf16 output (for cheap transposes)
                h_c = c_pool.tile([P, FREE], BF16, tag="h_c")
                for cb in range(CB):
                    cs = slice(cb * TC, (cb + 1) * TC)
                    if k == 0:
                        init = carry_sb[:, cb : cb + 1]
                    else:
                        init = h_prev[:, cb * TC + TC - 1 : cb * TC + TC]
                    nc.vector.tensor_tensor_scan(
                        h_c[:, cs], f_c[:, cs], i_c[:, cs], init, OP.mult, OP.add
                    )

                prev_tail = (k, h_c, og_t)
                h_prev = h_c

            # ---- drain the pipeline: last chunk's tail + h_last
            emit_tail(*prev_tail)
            hl_ps = ps_pool.tile([CB, P], BF16, tag="ps")
            nc.tensor.transpose(hl_ps[:], h_prev[:, TC - 1 :: TC], ident[:])
            hl_sb = const_pool.tile([CB, P], F32)
            nc.scalar.copy(hl_sb[:], hl_ps[:])
            nc.sync.dma_start(h_last[:, :], hl_sb[:])

    nc.compile()
    return nc


def _in_maps(x, carry):
    x = np.asarray(x, dtype=np.float32)
    carry = np.asarray(carry, dtype=np.float32)
    maps = []
    for core in range(8):
        b, half = core // 2, core % 2
        c0 = half * DHC
        maps.append(
            {
                "xin": np.ascontiguousarray(x[b, :, c0 : c0 + DHC]),
                "xig": np.ascontiguousarray(x[b, :, DH + c0 : DH + c0 + DHC]),
                "xog": np.ascontiguousarray(
                    x[b, :, 2 * DH + c0 : 2 * DH + c0 + DHC]
                ),
                "carry": np.ascontiguousarray(
                    carry[b, c0 : c0 + DHC].reshape(CB, P)
                ),
            }
        )
    return maps


def run(x, carry, trace=False, **kwargs):
    nc = _build()
    res = run_bass_kernel_spmd(
        nc, _in_maps(x, carry), core_ids=list(range(8)), trace=trace, **kwargs
    )
    y = np.zeros((B, S, DH), np.float32)
    hl = np.zeros((B, DH), np.float32)
    for core in range(8):
        b, half = core // 2, core % 2
        c0 = half * DHC
        y[b, :, c0 : c0 + DHC] = res.results[core]["y"]
        hl[b, c0 : c0 + DHC] = res.results[core]["h_last"].reshape(DHC)
    return (hl, y), res


def kernel(x, carry):
    out, _ = run(x, carry)
    return out
